# revision 18
# baseline (speedup 1.0000x reference)
"""Cross-attention Trainium2 kernel (8 NeuronCores).

Sharding: batch (2) x head-groups (4 groups of 4 heads) = 8 shards.
Each core computes q/k/v projections for its 4 heads (256 cols of
Wq/Wk/Wv), attention for those heads, and a partial out-projection
through its 256 rows of Wo.  The host sums the 4 partial outputs per
batch (the reduction of the head-parallel out_proj) and adds the
bv @ Wo + bo correction, which commutes exactly through the softmax
average.

Layout strategy on-core (v4):
  - x/ctx arrive HOST-TRANSPOSED (xT: [d, s]) so no PE transposes are
    needed; projections emit qT/kT ([head_dim, s], head pairs stacked
    even/odd on partitions 0-63/64-127) and v natural ([sk, head, hd]
    with a ones column for the softmax denominator).
  - scores are computed transposed (ST = k @ qT -> [sk, sq]); the
    even/odd head matmuls (K=64) are emitted adjacently so their
    auto-derived tile_positions (0,0)/(64,0) run concurrently on the
    two PE row-halves.
  - attention runs "orientation B": stationary = v(+ones) [sk,68],
    moving = the exp tile [sk,512].  The output IS attn^T [hd, sq]
    (what out_proj needs) with the softmax denominator in partition
    64; stationary loads are small and fully hidden under the 512-col
    moving streams.  Normalization = Pool partition_broadcast of the
    denominator row + DVE reciprocal + DVE multiply; odd heads reach
    partitions 64-127 through a tiny identity matmul (engines cannot
    write across partitions).
  - emission is paced by ACT (exp), the bottleneck engine (~134us
    busy): the prologue projects pair-0 columns first and spreads the
    first pair's scores between projection chunks so ACT starts at
    ~14us and never starves; each head-pair phase interleaves its
    attention matmuls with the NEXT pair's score matmuls (exp-tile
    liveness stays at one pair / 36 tiles); the last phase fuses both
    sq halves into one ACT-paced sweep (borrowing the projection PSUM
    pool) so only normalization + out_proj remain after the final exp.
"""

import numpy as np

import concourse.bass as bass
import concourse.mybir as mybir
import concourse.tile as tile
from concourse import bacc

B, SQ, SK, D, H, HS = 2, 2048, 2048, 1024, 16, 64
SCALE = HS ** -0.5
NCORES = 8
HG = 4            # heads per core
DG = HG * HS      # 256 projection cols per core

F32 = mybir.dt.float32
F16 = mybir.dt.float16
BF16 = mybir.dt.bfloat16


def build_program(loop_iters: int = 0):
    """Build the per-core SPMD Bass program."""
    nc = bacc.Bacc(None, target_bir_lowering=False, debug=False,
                   num_devices=NCORES)
    x_d = nc.dram_tensor("xT", [D, SQ], F16, kind="ExternalInput")
    c_d = nc.dram_tensor("cT", [D, SK], F16, kind="ExternalInput")
    wq_d = nc.dram_tensor("wq", [D, DG], F16, kind="ExternalInput")
    wk_d = nc.dram_tensor("wk", [D, DG], F16, kind="ExternalInput")
    wv_d = nc.dram_tensor("wv", [D, DG], F16, kind="ExternalInput")
    wo_d = nc.dram_tensor("wo", [DG, D], F16, kind="ExternalInput")
    bq_d = nc.dram_tensor("bq", [DG], F32, kind="ExternalInput")
    bk_d = nc.dram_tensor("bk", [DG], F32, kind="ExternalInput")
    i64_d = nc.dram_tensor("ident64", [64, 64], F16, kind="ExternalInput")
    out_d = nc.dram_tensor("out", [SQ, D], BF16, kind="ExternalOutput")

    with tile.TileContext(nc) as tc:
        with (
            tc.tile_pool(name="const", bufs=1) as cp,
            tc.tile_pool(name="persist", bufs=1) as psb,
            tc.tile_pool(name="xw", bufs=6) as xwp,
            tc.tile_pool(name="expp", bufs=38) as ep,
            tc.tile_pool(name="fin", bufs=6) as fpool,
            tc.tile_pool(name="outp", bufs=2) as opool,
            tc.tile_pool(name="pp", bufs=2, space="PSUM") as pp,
            tc.tile_pool(name="stp", bufs=2, space="PSUM") as stp,
            tc.tile_pool(name="atp", bufs=2, space="PSUM") as atp,
        ):
            import contextlib
            loop_ctx = tc.For_i(0, loop_iters, 1) if loop_iters else contextlib.nullcontext()
            loop_ctx.__enter__()

            ident64 = cp.tile([64, 64], F16, tag="ident64")
            wq_sb = cp.tile([128, 8, DG], F16, tag="wq")
            wk_sb = cp.tile([128, 8, DG], F16, tag="wk")
            wv_sb = cp.tile([128, 8, DG], F16, tag="wv")
            wo_sb = cp.tile([128, 2, D], F16, tag="wo")
            bq_sb = cp.tile([128, 2], F32, tag="bq")
            bk_sb = cp.tile([128, 2], F32, tag="bk")

            def load_weights_qx():
                nc.sync.dma_start(out=wq_sb, in_=wq_d[:].rearrange("(c p) n -> p c n", p=128))
                nc.sync.dma_start(out=bq_sb, in_=bq_d[:].rearrange("(c p) -> p c", p=128))

            def load_weights_k():
                nc.sync.dma_start(out=wk_sb, in_=wk_d[:].rearrange("(c p) n -> p c n", p=128))
                nc.sync.dma_start(out=bk_sb, in_=bk_d[:].rearrange("(c p) -> p c", p=128))
                nc.sync.dma_start(out=ident64, in_=i64_d[:])

            def load_weights_v():
                nc.sync.dma_start(out=wv_sb, in_=wv_d[:].rearrange("(c p) n -> p c n", p=128))

            def load_weights_o():
                nc.sync.dma_start(out=wo_sb, in_=wo_d[:].rearrange("(c p) n -> p c n", p=128))

            # persistent activations: qT/kT hold head pairs stacked on
            # partitions (even head p0-63, odd p64-127), pair index on the
            # middle axis, full sq/sk on the free axis
            qT = psb.tile([128, 2, SQ], F16, tag="qT", name="qT")
            kT = psb.tile([128, 2, SK], F16, tag="kT", name="kT")
            # v natural: [sk-chunk part, skc, head, 64+ones]
            vA = psb.tile([128, 16, HG, 68], F16, tag="vA", name="vA")
            # attn^T per sq-window: [pair-stacked head dim, pair, sq]
            aTw = [psb.tile([128, 2, 1024], F16, tag=f"aTw{s}", name=f"aTw{s}")
                   for s in range(2)]

            nc.vector.memset(vA[:], 1.0)

            cws = {}

            def proj_x(w, cs, dma=False, after_dma=None):
                if dma:
                    xw = xwp.tile([128, 8, 512], F16, tag="xw")
                    cws[("x", w)] = xw
                    nc.sync.dma_start(
                        out=xw,
                        in_=x_d[:, w * 512:(w + 1) * 512]
                            .rearrange("(c p) s -> p c s", p=128))
                    if after_dma is not None:
                        after_dma()
                xw = cws[("x", w)]
                for c in cs:
                    pq = pp.tile([128, 512], F32, tag="pp")
                    for dc in range(8):
                        nc.tensor.matmul(
                            pq,
                            (wq_sb[:, dc, c * 128:(c + 1) * 128]),
                            (xw[:, dc, :]),
                            start=(dc == 0), stop=(dc == 7),
                        )
                    nc.vector.tensor_scalar_add(
                        qT[:, c, w * 512:(w + 1) * 512], pq, bq_sb[:, c:c + 1])
                if cs[-1] == 1:
                    del cws[("x", w)]

            def proj_k(w, cs, dma=False, after_dma=None):
                if dma:
                    cw = xwp.tile([128, 8, 512], F16, tag="xw")
                    cws[("c", w)] = cw
                    nc.sync.dma_start(
                        out=cw,
                        in_=c_d[:, w * 512:(w + 1) * 512]
                            .rearrange("(c p) s -> p c s", p=128))
                    if after_dma is not None:
                        after_dma()
                cw = cws[("c", w)]
                for c in cs:
                    pk = pp.tile([128, 512], F32, tag="pp")
                    for dc in range(8):
                        nc.tensor.matmul(
                            pk,
                            (wk_sb[:, dc, c * 128:(c + 1) * 128]),
                            (cw[:, dc, :]),
                            start=(dc == 0), stop=(dc == 7),
                        )
                    nc.vector.tensor_scalar_add(
                        kT[:, c, w * 512:(w + 1) * 512], pk, bk_sb[:, c:c + 1])

            def proj_v(w, s4s):
                cw = cws[("c", w)]
                for s4 in s4s:
                    # runs inside phase-0 part1: projection psum pool is
                    # free there (atp holds the attn accumulators)
                    pv = pp.tile([128, 512], F32, tag="pp")
                    for dc in range(8):
                        nc.tensor.matmul(
                            pv[:, :DG],
                            (cw[:, dc, s4 * 128:(s4 + 1) * 128]),
                            (wv_sb[:, dc, :]),
                            start=(dc == 0), stop=(dc == 7),
                        )
                    nc.vector.tensor_copy(
                        vA[:, w * 4 + s4, :, 0:64],
                        pv[:, :DG].rearrange("p (h e) -> p h e", e=64),
                    )
                if s4s[-1] == 3:
                    del cws[("c", w)]

            # scores + exp for head pair t, sq window sqw, one sk chunk.
            # even/odd half-array matmuls are emitted adjacently so their
            # tile_positions (0,0)/(64,0) run concurrently on HW.
            def emit_se(t, sqw, skc):
                sts = [stp.tile([128, 1024], F32, tag="st", name=f"st{p}")
                       for p in range(2)]
                for half in range(2):
                    for par in range(2):
                        p0 = 64 * par
                        nc.tensor.matmul(
                            sts[par][:, half * 512:(half + 1) * 512],
                            (kT[p0:p0 + 64, t, skc * 128:(skc + 1) * 128]),
                            (qT[p0:p0 + 64, t,
                                sqw * 1024 + half * 512:
                                sqw * 1024 + (half + 1) * 512]),
                            start=True, stop=True,
                        )
                exs = []
                for par in range(2):
                    ex = ep.tile([128, 1024], F16, tag="ex")
                    nc.scalar.activation(
                        ex, sts[par], mybir.ActivationFunctionType.Exp,
                        scale=SCALE)
                    exs.append(ex)
                return exs

            # one attention accumulation matmul: stationary v(+ones),
            # moving a 512-wide half of the exp tile
            def mm_at(at, ex, h, skc, half):
                nc.tensor.matmul(
                    at[0:68, :],
                    vA[:, skc, h, :],
                    ex[:, half * 512:(half + 1) * 512],
                    start=(skc == 0), stop=(skc == 15),
                )

            # normalize one head's attn^T half and place it in aTw
            def norm(t, sqw, par, half, at, shift_pool=None):
                # DVE reciprocal of the denominator row (Pool cannot read
                # PSUM), then Pool broadcasts it across the 64 partitions
                rcrow = fpool.tile([1, 512], F32, tag="rcrow")
                nc.vector.reciprocal(rcrow, at[64:65, :])
                rc = fpool.tile([64, 512], F32, tag="rc")
                nc.gpsimd.partition_broadcast(rc, rcrow)
                dst = aTw[sqw][64 * par:64 * par + 64, t,
                               half * 512:(half + 1) * 512]
                if par == 0:
                    nc.vector.tensor_mul(dst, at[0:64, :], rc)
                else:
                    # engines cannot shift partitions; bounce through the
                    # PE with a small identity matmul
                    tmp = fpool.tile([64, 512], F16, tag="atmp")
                    nc.vector.tensor_mul(tmp, at[0:64, :], rc)
                    sp, sptag = shift_pool or (pp, "pp")
                    ps = sp.tile([128, 512], F32, tag=sptag, name="ps")
                    nc.tensor.matmul(ps[64:128, :], ident64, tmp,
                                     start=True, stop=True)
                    nc.vector.tensor_copy(dst, ps[64:128, :])

            # partial out-projection for one 128-row sq chunk; in the tail
            # (after the last exp) ACT is idle, so split the PSUM drains
            # between DVE and ScalarE there
            def emit_out_proj(sqc, use_act=False, po_pool=None):
                ot = opool.tile([128, D], BF16, tag="ot")
                sqw, c8 = sqc // 8, sqc % 8
                opl, optag = po_pool or (pp, "pp")
                for n2 in range(2):
                    po = opl.tile([128, 512], F32, tag=optag)
                    for kc in range(2):
                        nc.tensor.matmul(
                            po,
                            (aTw[sqw][:, kc, c8 * 128:(c8 + 1) * 128]),
                            (wo_sb[:, kc, n2 * 512:(n2 + 1) * 512]),
                            start=(kc == 0), stop=(kc == 1),
                        )
                    if use_act and n2 == 1:
                        nc.scalar.copy(ot[:, n2 * 512:(n2 + 1) * 512], po)
                    else:
                        nc.vector.tensor_copy(
                            ot[:, n2 * 512:(n2 + 1) * 512], po)
                nc.sync.dma_start(
                    out=out_d[sqc * 128:(sqc + 1) * 128, :], in_=ot)

            # ---- prologue: pair-0 projection columns first, with the
            # first pair's scores spread between projection chunks so ACT
            # starts early and stays fed
            P = [(0, 0), (1, 0), (0, 1), (1, 1)]
            e = {}
            se0 = []
            proj_x(0, [0], dma=True, after_dma=load_weights_qx)
            proj_k(0, [0], dma=True, after_dma=load_weights_k)
            proj_x(1, [0], dma=True)
            se0.append(emit_se(0, 0, 0))
            se0.append(emit_se(0, 0, 1))
            proj_x(0, [1])
            se0.append(emit_se(0, 0, 2))
            proj_x(1, [1])
            se0.append(emit_se(0, 0, 3))
            proj_k(1, [0], dma=True, after_dma=load_weights_v)
            se0.append(emit_se(0, 0, 4))
            proj_k(0, [1])
            se0.append(emit_se(0, 0, 5))
            proj_x(2, [0, 1], dma=True)
            se0.append(emit_se(0, 0, 6))
            proj_k(2, [0], dma=True)
            se0.append(emit_se(0, 0, 7))
            proj_k(1, [1])
            se0.append(emit_se(0, 0, 8))
            proj_x(3, [0, 1], dma=True)
            se0.append(emit_se(0, 0, 9))
            proj_k(3, [0], dma=True, after_dma=load_weights_o)
            se0.append(emit_se(0, 0, 10))
            proj_k(2, [1])
            se0.append(emit_se(0, 0, 11))
            proj_k(3, [1])
            se0 += [emit_se(0, 0, j) for j in range(12, 16)]
            e[P[0]] = se0
            # pre-emit the next pair's first scores so ACT rolls from P0's
            # exps straight into P1's while phase 0 catches up on PE
            e[P[1]] = [emit_se(1, 0, j) for j in range(3)]

            # ---- main: 4 head-pair phases, ACT-paced
            for i in range(4):
                t, sqw = P[i]
                exE = [a for a, _ in e[P[i]]]
                exO = [b for _, b in e[P[i]]]
                atE = atp.tile([128, 512], F32, tag="at", name="atE")
                atO = atp.tile([128, 512], F32, tag="at", name="atO")
                if i < 3:
                    # part 1: first sq half, ACT-paced.  Phase 0 weaves the
                    # v-projections into the exp-gated idle slots.
                    for skc in range(16):
                        if i == 0:
                            proj_v(skc // 4, [skc % 4])
                        mm_at(atE, exE[skc], 2 * t, skc, 0)
                        mm_at(atO, exO[skc], 2 * t + 1, skc, 0)
                    norm(t, sqw, 0, 0, atE)
                    norm(t, sqw, 1, 0, atO)
                    se = e.setdefault(P[i + 1], [])
                    while len(se) < 3:
                        se.append(emit_se(*P[i + 1], len(se)))
                    if i == 1:
                        for sqc in range(0, 4):
                            emit_out_proj(sqc)
                    # part 2: second sq half, weaving the next pair's scores
                    atE2 = atp.tile([128, 512], F32, tag="at", name="atE2")
                    atO2 = atp.tile([128, 512], F32, tag="at", name="atO2")
                    for skc in range(16):
                        mm_at(atE2, exE[skc], 2 * t, skc, 1)
                        mm_at(atO2, exO[skc], 2 * t + 1, skc, 1)
                        if len(se) < 16:
                            se.append(emit_se(*P[i + 1], len(se)))
                    norm(t, sqw, 0, 1, atE2)
                    norm(t, sqw, 1, 1, atO2)
                    if i == 1:
                        for sqc in range(4, 8):
                            emit_out_proj(sqc)
                else:
                    # final phase: both sq halves in one ACT-paced sweep
                    # (the projection PSUM pool is free here), so only
                    # normalization + out_proj trail the last exp
                    atE2 = pp.tile([128, 512], F32, tag="pp", name="atE2")
                    atO2 = pp.tile([128, 512], F32, tag="pp", name="atO2")
                    for skc in range(16):
                        mm_at(atE, exE[skc], 2 * t, skc, 0)
                        mm_at(atO, exO[skc], 2 * t + 1, skc, 0)
                        mm_at(atE2, exE[skc], 2 * t, skc, 1)
                        mm_at(atO2, exO[skc], 2 * t + 1, skc, 1)
                    # half-a norms first (shift psum + out_proj psums borrow
                    # the freed atp slots) so out_proj(8..11) overlaps the
                    # half-b norms
                    norm(t, sqw, 0, 0, atE)
                    norm(t, sqw, 1, 0, atO, shift_pool=(atp, "at"))
                    for sqc in range(8, 12):
                        emit_out_proj(sqc, use_act=True, po_pool=(atp, "at"))
                    norm(t, sqw, 0, 1, atE2)
                    norm(t, sqw, 1, 1, atO2)
                    for sqc in range(12, 16):
                        emit_out_proj(sqc, use_act=True)
            loop_ctx.__exit__(None, None, None)

    nc.compile()
    return nc


_NC = None


def _program():
    global _NC
    if _NC is None:
        _NC = build_program()
    return _NC


def _f32(a):
    return np.ascontiguousarray(np.asarray(a, dtype=np.float32))


def make_in_maps(inputs, context, Wq, bq, Wk, bk, Wv, bv, Wo, bo):
    inputs = np.asarray(inputs)
    context = np.asarray(context)
    Wq, bq, Wk, bk = (np.asarray(a) for a in (Wq, bq, Wk, bk))
    Wv, Wo = np.asarray(Wv), np.asarray(Wo)
    in_maps = []
    for core in range(NCORES):
        b, g = core // HG, core % HG
        sl = slice(DG * g, DG * (g + 1))
        in_maps.append({
            "xT": np.ascontiguousarray(inputs[b].T.astype(np.float16)),
            "cT": np.ascontiguousarray(context[b].T.astype(np.float16)),
            "wq": np.ascontiguousarray(Wq[:, sl].astype(np.float16)),
            "wk": np.ascontiguousarray(Wk[:, sl].astype(np.float16)),
            "wv": np.ascontiguousarray(Wv[:, sl].astype(np.float16)),
            "wo": np.ascontiguousarray(Wo[sl, :].astype(np.float16)),
            "bq": _f32(bq[sl]),
            "bk": _f32(bk[sl]),
            "ident64": np.eye(64, dtype=np.float16),
        })
    return in_maps


def kernel(inputs, context, Wq, bq, Wk, bk, Wv, bv, Wo, bo):
    from concourse.bass_utils import run_bass_kernel_spmd

    nc = _program()
    in_maps = make_in_maps(inputs, context, Wq, bq, Wk, bk, Wv, bv, Wo, bo)
    res = run_bass_kernel_spmd(nc, in_maps, list(range(NCORES)))
    outs = [np.asarray(res.results[i]["out"]).astype(np.float32)
            for i in range(NCORES)]
    bv = _f32(bv)
    Wo = _f32(Wo)
    bo = _f32(bo)
    corr = (bv.astype(np.float64) @ Wo.astype(np.float64)
            + bo.astype(np.float64)).astype(np.float32)
    full = np.stack([
        outs[0] + outs[1] + outs[2] + outs[3],
        outs[4] + outs[5] + outs[6] + outs[7],
    ]) + corr
    return full.astype(np.float32)
